# revision 7
# baseline (speedup 1.0000x reference)
"""EdgeDegreeEmbedding Trainium2 kernel (8 NeuronCores, SPMD, no collectives).

Strategy: shard by TARGET NODE (625 nodes/core). Host sorts edges by target
and packs each node's edges into one 18-edge "slot"; a slot's message sum is
a single PE matmul with K = 18 edges x 7 m-coeffs = 126 contraction rows
(wigner rows stacked along K, envelope/RESCALE pre-folded). Nodes with more
than 18 edges spill into overflow slots that the host adds back at the end.
Each core therefore scatter-adds only into its private node range -> the
per-core outputs are disjoint shards, no allreduce.
"""

import numpy as np

import concourse.bass as bass
import concourse.mybir as mybir
from concourse import tile
from concourse.bass_utils import run_bass_kernel_spmd
from concourse.vector_clock import ScopedClock

# ---- problem constants (hardcoded; must match the reference) ----
SPHERE = 128
M0 = 7
LFULL = 49
CUTOFF = 12.0
RESCALE = 23.395238876342773
LN_EPS = 1e-5
N_NODES, N_EDGES, D_DIST = 5000, 50000, 512

N_CORES = 8
NODES_PER_CORE = N_NODES // N_CORES  # 625
SLOT_E = 18              # edges per slot
ROWS = SLOT_E * M0       # 126 contraction rows per slot
TILE_SLOTS = 7           # slots per MLP tile
TILE_E = SLOT_E * TILE_SLOTS  # 126 edges per tile
S_MAIN = 630             # 625 real nodes + 5 dummies -> multiple of 7
T_MAIN = S_MAIN // TILE_SLOTS  # 90

BF16 = mybir.dt.bfloat16
F32 = mybir.dt.float32
NP_BF16 = mybir.dt.np(BF16)

_CACHE = {}
TRACE = False      # set True (e.g. from test.py) to profile the run
TRACE_KW = {}      # extra kwargs for run_bass_kernel_spmd when tracing
LAST = None        # BassKernelResults of the most recent run


class _ChunkedDrainTC(tile.TileContext):
    """Walrus here rejects >1 sync wait per instruction; spread every
    multi-wait instruction's extras over preceding same-engine nops, and do
    the same for the Tile exit-drain's global-clock waits."""

    def _lower_ordered_insts(self, ordered):
        for bb_name, insts in ordered.items():
            out = []
            for inst in insts:
                si = getattr(inst, "sync_info", None)
                waits = list(si.on_wait) if si is not None and si.on_wait else []
                if len(waits) > 1 and type(inst).__name__.startswith("Inst"):
                    for w in waits[:-1]:
                        out.append(mybir.InstNoOp(
                            name=self.nc.get_next_instruction_name(),
                            sync_info=mybir.SyncInfo(on_wait=[w], on_update=[]),
                            bass_nofuse=True,
                            engine=inst.engine,
                        ))
                    si.on_wait = waits[-1:]
                out.append(inst)
            ordered[bb_name] = out
        return super()._lower_ordered_insts(ordered)

    def _drain_and_barrier(self, tick_clock, wait_clock):
        nc = self.nc
        probe = nc.sync.nop()
        wait_clock.add_sem_waits(
            probe.ins, ScopedClock({None: tick_clock.global_clock})
        )
        si = probe.ins.sync_info
        waits = list(si.on_wait) if si and si.on_wait else []
        si.on_wait = waits[:1]
        for w in waits[1:]:
            n2 = nc.sync.nop()
            n2.ins.sync_info = mybir.SyncInfo(on_wait=[w], on_update=[])
        nc.sync.drain()
        nc.all_engine_barrier()
        popped = nc._tile_sem_poison_stack.pop()
        assert popped is self._sem_poison
        nc.clear_and_free_semaphores(list(self.sems.allocated().values()))
        nc.all_engine_barrier()


def _build_nc(T, S_OV):
    """Build the SPMD Bass program for T tiles total (T_MAIN main tiles plus
    T-T_MAIN overflow tiles; S_OV = overflow slots)."""
    T_OV = T - T_MAIN
    nc = bass.Bass("TRN2", target_bir_lowering=False, num_devices=N_CORES)

    xeT = nc.dram_tensor("xeT", [T, 128, 6 * TILE_E], BF16, kind="ExternalInput")
    wst = nc.dram_tensor("wst", [T, 128, TILE_SLOTS * LFULL], BF16,
                         kind="ExternalInput")
    xr = nc.dram_tensor("xr", [T_MAIN, LFULL, TILE_SLOTS * 128], F32,
                        kind="ExternalInput")
    w1 = nc.dram_tensor("w1", [128, 6 * 128], BF16, kind="ExternalInput")
    w2 = nc.dram_tensor("w2", [128, 128], BF16, kind="ExternalInput")
    w3 = nc.dram_tensor("w3", [128, M0 * SPHERE], BF16, kind="ExternalInput")
    ident = nc.dram_tensor("ident", [TILE_E, TILE_E], BF16, kind="ExternalInput")

    outr = nc.dram_tensor("outr", [T_MAIN, LFULL, TILE_SLOTS * 128], F32,
                          kind="ExternalOutput")
    ovr = nc.dram_tensor("ovr", [T_OV, LFULL, TILE_SLOTS * 128], F32,
                         kind="ExternalOutput")

    with _ChunkedDrainTC(nc) as tc:
        with (
            tc.tile_pool(name="const", bufs=1) as cpool,
            tc.tile_pool(name="xe", bufs=3) as xe_pool,
            tc.tile_pool(name="wt", bufs=3) as w_pool,
            tc.tile_pool(name="xt", bufs=3) as x_pool,
            tc.tile_pool(name="h", bufs=3) as h_pool,
            tc.tile_pool(name="m0", bufs=3) as m0_pool,
            tc.tile_pool(name="stk", bufs=3) as st_pool,
            tc.tile_pool(name="outt", bufs=3) as out_pool,
            tc.tile_pool(name="stat", bufs=4) as stat_pool,
            tc.tile_pool(name="ps", bufs=3, space="PSUM") as ps_pool,
            tc.tile_pool(name="psr", bufs=4, space="PSUM") as psr_pool,
        ):
            w1_sb = cpool.tile([128, 6 * 128], BF16)
            nc.sync.dma_start(w1_sb[:], w1[:])
            w2_sb = cpool.tile([128, 128], BF16)
            nc.sync.dma_start(w2_sb[:], w2[:])
            w3_sb = cpool.tile([128, M0 * SPHERE], BF16)
            nc.sync.dma_start(w3_sb[:], w3[:])
            id_sb = cpool.tile([TILE_E, TILE_E], BF16)
            nc.sync.dma_start(id_sb[:], ident[:])

            def layernorm_silu(ps, h_out):
                st = stat_pool.tile([TILE_E, 6], F32, tag="bn")
                nc.vector.bn_stats(st[:], ps[:])
                mv = stat_pool.tile([TILE_E, 2], F32, tag="mv")
                nc.vector.bn_aggr(mv[:], st[:])
                inv = stat_pool.tile([TILE_E, 1], F32, tag="inv")
                nc.vector.tensor_scalar_add(inv[:], mv[:, 1:2], LN_EPS)
                sd = stat_pool.tile([TILE_E, 1], F32, tag="sd")
                nc.scalar.activation(sd[:], inv[:],
                                     mybir.ActivationFunctionType.Sqrt)
                nc.vector.reciprocal(inv[:], sd[:])
                nmi = stat_pool.tile([TILE_E, 1], F32, tag="nmi")
                nc.vector.tensor_mul(nmi[:], mv[:, 0:1], inv[:])
                nc.vector.tensor_scalar_mul(nmi[:], nmi[:], -1.0)
                nc.scalar.activation(h_out[:], ps[:],
                                     mybir.ActivationFunctionType.Silu,
                                     bias=nmi[:], scale=inv[:])

            for t in range(T):
                is_main = t < T_MAIN
                xe_t = xe_pool.tile([128, 6 * TILE_E], BF16)
                nc.sync.dma_start(xe_t[:], xeT[t])
                w_t = w_pool.tile([128, TILE_SLOTS * LFULL], BF16)
                nc.sync.dma_start(w_t[:], wst[t])
                if is_main:
                    x_t = x_pool.tile([LFULL, TILE_SLOTS * 128], F32)
                    nc.sync.dma_start(x_t[:], xr[t])

                # MLP layer 1: [126e,768] @ [768,128] -> psum [126e,128ch]
                ps1 = ps_pool.tile([TILE_E, 448], F32, tag="ps")
                for k in range(6):
                    nc.tensor.matmul(
                        ps1[:, 0:128],
                        xe_t[:, k * TILE_E:(k + 1) * TILE_E],
                        w1_sb[:, k * 128:(k + 1) * 128],
                        start=(k == 0), stop=(k == 5),
                    )
                h1 = h_pool.tile([TILE_E, 128], BF16, tag="h")
                layernorm_silu(ps1[:, 0:128], h1)

                pst1 = ps_pool.tile([128, 448], BF16, tag="ps")
                nc.tensor.transpose(pst1[:, 0:TILE_E], h1[:], id_sb[:])
                h1t = h_pool.tile([128, TILE_E], BF16, tag="ht")
                nc.vector.tensor_copy(h1t[:], pst1[:, 0:TILE_E])

                # MLP layer 2
                ps2 = ps_pool.tile([TILE_E, 448], F32, tag="ps")
                nc.tensor.matmul(ps2[:, 0:128], h1t[:], w2_sb[:],
                                 start=True, stop=True)
                h2 = h_pool.tile([TILE_E, 128], BF16, tag="h")
                layernorm_silu(ps2[:, 0:128], h2)

                pst2 = ps_pool.tile([128, 448], BF16, tag="ps")
                nc.tensor.transpose(pst2[:, 0:TILE_E], h2[:], id_sb[:])
                h2t = h_pool.tile([128, TILE_E], BF16, tag="ht")
                nc.vector.tensor_copy(h2t[:], pst2[:, 0:TILE_E])

                # MLP layer 3: -> m0 [126e, 896]
                m0a = ps_pool.tile([TILE_E, 448], F32, tag="ps")
                nc.tensor.matmul(m0a[:], h2t[:], w3_sb[:, 0:448],
                                 start=True, stop=True)
                m0b = ps_pool.tile([TILE_E, 448], F32, tag="ps")
                nc.tensor.matmul(m0b[:], h2t[:], w3_sb[:, 448:896],
                                 start=True, stop=True)
                m0_sb = m0_pool.tile([TILE_E, M0 * SPHERE], BF16)
                nc.vector.tensor_copy(m0_sb[:, 0:448], m0a[:])
                nc.vector.tensor_copy(m0_sb[:, 448:896], m0b[:])

                # reshape: stacked[7i+m, c] = m0[18j+i, 128m+c] per slot j
                stacked = st_pool.tile([128, TILE_SLOTS, 128], BF16)
                for j in range(TILE_SLOTS):
                    nc.sync.dma_start(
                        stacked[0:ROWS, j, :],
                        m0_sb[j * SLOT_E:(j + 1) * SLOT_E, :],
                    )

                # rotation + node accumulate: one matmul per slot (=node)
                out_sb = out_pool.tile([LFULL, TILE_SLOTS, 128], F32)
                for j in range(TILE_SLOTS):
                    rot = psr_pool.tile([LFULL, 128], F32, tag="rot")
                    nc.tensor.matmul(
                        rot[:],
                        w_t[0:ROWS, j * LFULL:(j + 1) * LFULL],
                        stacked[0:ROWS, j, :],
                        start=True, stop=True,
                    )
                    if is_main:
                        nc.vector.tensor_add(
                            out_sb[:, j, :], rot[:],
                            x_t[:, j * 128:(j + 1) * 128],
                        )
                    else:
                        nc.vector.tensor_copy(out_sb[:, j, :], rot[:])

                if is_main:
                    nc.sync.dma_start(outr[t], out_sb[:])
                else:
                    nc.sync.dma_start(ovr[t - T_MAIN], out_sb[:])

    return nc


def _envelope(d):
    e = 1.0 + (-21.0) * d ** 5 + 35.0 * d ** 6 + (-15.0) * d ** 7
    return np.where(d < 1.0, e, 0.0)


def kernel(**inputs):
    x = np.asarray(inputs["x"], np.float32)
    dist_emb = np.asarray(inputs["edge_distance_embedding"], np.float32)
    src_emb = np.asarray(inputs["source_atom_embedding"], np.float32)
    tgt_emb = np.asarray(inputs["target_atom_embedding"], np.float32)
    edge_distance = np.asarray(inputs["edge_distance"], np.float64)
    edge_index = np.asarray(inputs["edge_index"]).astype(np.int64)
    wigner = np.asarray(inputs["wigner_and_M_mapping_inv"], np.float32)
    W1 = np.asarray(inputs["W1"], np.float32)
    W2 = np.asarray(inputs["W2"], np.float32)
    W3 = np.asarray(inputs["W3"], np.float32)
    # biases/gains are zeros/ones by construction; folded out of the kernel
    for nm, triv in (("b1", 0), ("bt1", 0), ("b2", 0), ("bt2", 0), ("b3", 0),
                     ("g1", 1), ("g2", 1)):
        v = np.asarray(inputs[nm])
        assert np.all(v == triv), f"{nm} not trivial; unsupported fast path"

    srcs, tgts = edge_index[0], edge_index[1]
    scale = (_envelope(edge_distance / CUTOFF) / RESCALE).astype(np.float32)

    order = np.argsort(tgts, kind="stable")
    tsorted = tgts[order]
    # edges of node n: order[starts[n]:starts[n+1]]
    starts = np.searchsorted(tsorted, np.arange(N_NODES + 1))

    # ---- build slots per core ----
    core_slots = []  # per core: list of (node_local, edge_id array)
    max_ov = 0
    for c in range(N_CORES):
        slots_main = []
        slots_ov = []
        base = c * NODES_PER_CORE
        for nl in range(NODES_PER_CORE):
            eids = order[starts[base + nl]:starts[base + nl + 1]]
            slots_main.append((nl, eids[:SLOT_E]))
            rest = eids[SLOT_E:]
            while len(rest) > 0:
                slots_ov.append((nl, rest[:SLOT_E]))
                rest = rest[SLOT_E:]
        for nl in range(NODES_PER_CORE, S_MAIN):
            slots_main.append((nl, np.empty(0, np.int64)))  # dummy
        core_slots.append((slots_main, slots_ov))
        max_ov = max(max_ov, len(slots_ov))

    S_OV = max(TILE_SLOTS, -(-max_ov // TILE_SLOTS) * TILE_SLOTS)
    S = S_MAIN + S_OV
    T = S // TILE_SLOTS
    E_pad = S * SLOT_E

    key = (T, S_OV)
    if key not in _CACHE:
        _CACHE[key] = _build_nc(T, S_OV)
    nc = _CACHE[key]

    # ---- shared weight tensors ----
    w1_in = np.ascontiguousarray(
        W1.reshape(6, 128, 128).transpose(1, 0, 2).reshape(128, 6 * 128)
    ).astype(NP_BF16)
    w2_in = W2.astype(NP_BF16)
    w3_in = W3.astype(NP_BF16)
    ident = np.eye(TILE_E, dtype=np.float32).astype(NP_BF16)

    in_maps = []
    ov_maps = []  # per core: list of node_local per overflow slot
    for c in range(N_CORES):
        slots_main, slots_ov = core_slots[c]
        slots = slots_main + slots_ov + [
            (0, np.empty(0, np.int64))
        ] * (S_OV - len(slots_ov))

        eorder = np.full(E_pad, -1, np.int64)
        for s, (_, eids) in enumerate(slots):
            eorder[s * SLOT_E:s * SLOT_E + len(eids)] = eids
        valid = eorder >= 0
        idx = eorder[valid]

        # xe gather -> [E_pad, 768] -> tiled bf16 [T, 128, 6*126]
        xe = np.zeros((E_pad, 768), np.float32)
        xe[valid, :D_DIST] = dist_emb[idx]
        xe[valid, D_DIST:D_DIST + 128] = src_emb[srcs[idx]]
        xe[valid, D_DIST + 128:] = tgt_emb[tgts[idx]]
        # xeT[p, k, e] = xe[e, 128k+p]; tile: [T, 128, 6, 126] -> [T,128,756]
        xeT = xe.reshape(T, TILE_E, 6, 128).transpose(0, 3, 2, 1)
        xe_in = np.ascontiguousarray(
            xeT.reshape(T, 128, 6 * TILE_E)).astype(NP_BF16)

        # wigner stack: [S, 128 rows, 49] -> [T, 128, 7*49]
        wrows = np.zeros((E_pad, M0, LFULL), np.float32)
        wrows[valid] = (
            wigner[idx, :, :M0] * scale[idx][:, None, None]
        ).transpose(0, 2, 1)
        wrows = wrows.reshape(S, ROWS, LFULL)
        wpad = np.zeros((S, 128, LFULL), np.float32)
        wpad[:, :ROWS] = wrows
        w_in = np.ascontiguousarray(
            wpad.reshape(T, TILE_SLOTS, 128, LFULL)
            .transpose(0, 2, 1, 3)
            .reshape(T, 128, TILE_SLOTS * LFULL)
        ).astype(NP_BF16)

        # x shard rearranged: [T_MAIN, 49, 7*128]
        xs = np.zeros((S_MAIN, LFULL, 128), np.float32)
        xs[:NODES_PER_CORE] = x[c * NODES_PER_CORE:(c + 1) * NODES_PER_CORE]
        x_in = np.ascontiguousarray(
            xs.reshape(T_MAIN, TILE_SLOTS, LFULL, 128)
            .transpose(0, 2, 1, 3)
            .reshape(T_MAIN, LFULL, TILE_SLOTS * 128)
        )

        in_maps.append({
            "xeT": xe_in, "wst": w_in, "xr": x_in,
            "w1": w1_in, "w2": w2_in, "w3": w3_in, "ident": ident,
        })
        ov_maps.append([nl for nl, _ in slots_ov])

    global LAST
    res = run_bass_kernel_spmd(
        nc, in_maps, core_ids=list(range(N_CORES)), trace=TRACE, **TRACE_KW
    )
    LAST = res

    out = np.empty((N_NODES, LFULL, SPHERE), np.float32)
    for c in range(N_CORES):
        r = res.results[c]
        o = np.asarray(r["outr"], np.float32).reshape(
            T_MAIN, LFULL, TILE_SLOTS, 128).transpose(0, 2, 1, 3).reshape(
            S_MAIN, LFULL, 128)
        oc = o[:NODES_PER_CORE]
        ov = np.asarray(r["ovr"], np.float32).reshape(
            T - T_MAIN, LFULL, TILE_SLOTS, 128).transpose(0, 2, 1, 3).reshape(
            S_OV, LFULL, 128)
        for s, nl in enumerate(ov_maps[c]):
            oc[nl] += ov[s]
        out[c * NODES_PER_CORE:(c + 1) * NODES_PER_CORE] = oc
    return out


# revision 8
# speedup vs baseline: 1.3414x; 1.3414x over previous
"""EdgeDegreeEmbedding Trainium2 kernel (8 NeuronCores, SPMD, no collectives).

Strategy: shard by TARGET NODE (625 nodes/core). Host sorts edges by target
and packs each node's edges into one 18-edge "slot"; a slot's message sum is
a single PE matmul with K = 18 edges x 7 m-coeffs = 126 contraction rows
(wigner rows stacked along K, envelope/RESCALE pre-folded). Nodes with more
than 18 edges spill into overflow slots that the host adds back at the end.
Each core therefore scatter-adds only into its private node range -> the
per-core outputs are disjoint shards, no allreduce.

The rotation matmul keeps the stacked m0 as the stationary operand
(M=128 channels, FWL-eligible) and streams the wigner slot (N=49), so the
node result lands transposed [channel, freq]; the host transposes back.
LayerNorm uses bn_stats + a quake-seeded Newton rsqrt (DVE+GpSimd) so the
scalar engine only ever loads the Silu table.
"""

import numpy as np

import concourse.bass as bass
import concourse.mybir as mybir
from concourse import tile
from concourse.bass_utils import run_bass_kernel_spmd
from concourse.vector_clock import ScopedClock

# ---- problem constants (hardcoded; must match the reference) ----
SPHERE = 128
M0 = 7
LFULL = 49
CUTOFF = 12.0
RESCALE = 23.395238876342773
LN_EPS = 1e-5
N_NODES, N_EDGES, D_DIST = 5000, 50000, 512

N_CORES = 8
NODES_PER_CORE = N_NODES // N_CORES  # 625
SLOT_E = 18              # edges per slot
ROWS = SLOT_E * M0       # 126 contraction rows per slot
TILE_SLOTS = 7           # slots per MLP tile
TILE_E = SLOT_E * TILE_SLOTS  # 126 edges per tile
S_MAIN = 630             # 625 real nodes + 5 dummies -> multiple of 7
T_MAIN = S_MAIN // TILE_SLOTS  # 90
XWF = 6 * 128 + TILE_SLOTS * LFULL  # 768 + 343 = 1111
RMAGIC = 0x5F3759DF

BF16 = mybir.dt.bfloat16
F32 = mybir.dt.float32
I32 = mybir.dt.int32
NP_BF16 = mybir.dt.np(BF16)

_CACHE = {}
TRACE = False      # set True (e.g. from test.py) to profile the run
TRACE_KW = {}      # extra kwargs for run_bass_kernel_spmd when tracing
LAST = None        # BassKernelResults of the most recent run


class _ChunkedDrainTC(tile.TileContext):
    """Walrus here rejects >1 sync wait per instruction; spread every
    multi-wait instruction's extras over preceding same-engine nops, and do
    the same for the Tile exit-drain's global-clock waits."""

    def _lower_ordered_insts(self, ordered):
        for bb_name, insts in ordered.items():
            out = []
            for inst in insts:
                si = getattr(inst, "sync_info", None)
                waits = list(si.on_wait) if si is not None and si.on_wait else []
                if len(waits) > 1 and type(inst).__name__.startswith("Inst"):
                    for w in waits[:-1]:
                        out.append(mybir.InstNoOp(
                            name=self.nc.get_next_instruction_name(),
                            sync_info=mybir.SyncInfo(on_wait=[w], on_update=[]),
                            bass_nofuse=True,
                            engine=inst.engine,
                        ))
                    si.on_wait = waits[-1:]
                out.append(inst)
            ordered[bb_name] = out
        return super()._lower_ordered_insts(ordered)

    def _drain_and_barrier(self, tick_clock, wait_clock):
        nc = self.nc
        probe = nc.sync.nop()
        wait_clock.add_sem_waits(
            probe.ins, ScopedClock({None: tick_clock.global_clock})
        )
        si = probe.ins.sync_info
        waits = list(si.on_wait) if si and si.on_wait else []
        si.on_wait = waits[:1]
        for w in waits[1:]:
            n2 = nc.sync.nop()
            n2.ins.sync_info = mybir.SyncInfo(on_wait=[w], on_update=[])
        nc.sync.drain()
        nc.all_engine_barrier()
        popped = nc._tile_sem_poison_stack.pop()
        assert popped is self._sem_poison
        nc.clear_and_free_semaphores(list(self.sems.allocated().values()))
        nc.all_engine_barrier()


def _build_nc(T, S_OV):
    """Build the SPMD Bass program for T tiles total (T_MAIN main tiles plus
    T-T_MAIN overflow tiles; S_OV = overflow slots)."""
    T_OV = T - T_MAIN
    nc = bass.Bass("TRN2", target_bir_lowering=False, num_devices=N_CORES)

    xw = nc.dram_tensor("xw", [T, 128, XWF], BF16, kind="ExternalInput")
    xr = nc.dram_tensor("xr", [T_MAIN, 128, TILE_SLOTS * LFULL], F32,
                        kind="ExternalInput")
    w1 = nc.dram_tensor("w1", [128, 6 * 128], BF16, kind="ExternalInput")
    w2 = nc.dram_tensor("w2", [128, 128], BF16, kind="ExternalInput")
    w3 = nc.dram_tensor("w3", [128, M0 * SPHERE], BF16, kind="ExternalInput")
    ident = nc.dram_tensor("ident", [128, 128], BF16, kind="ExternalInput")

    outr = nc.dram_tensor("outr", [T_MAIN, 128, TILE_SLOTS * LFULL], F32,
                          kind="ExternalOutput")
    ovr = nc.dram_tensor("ovr", [T_OV, 128, TILE_SLOTS * LFULL], F32,
                         kind="ExternalOutput")

    # round-robin DMA issue over both HWDGE rings (SP + ACT sequencers)
    ring_state = [0]

    def dma(dst, src):
        eng = nc.sync if ring_state[0] % 2 == 0 else nc.scalar
        ring_state[0] += 1
        eng.dma_start(dst, src)

    with _ChunkedDrainTC(nc) as tc:
        with (
            tc.tile_pool(name="const", bufs=1) as cpool,
            tc.tile_pool(name="xw", bufs=4) as xw_pool,
            tc.tile_pool(name="xt", bufs=3) as x_pool,
            tc.tile_pool(name="h", bufs=3) as h_pool,
            tc.tile_pool(name="m0", bufs=3) as m0_pool,
            tc.tile_pool(name="stk", bufs=3) as st_pool,
            tc.tile_pool(name="outt", bufs=3) as out_pool,
            tc.tile_pool(name="stat", bufs=4) as stat_pool,
            tc.tile_pool(name="ps", bufs=3, space="PSUM") as ps_pool,
            tc.tile_pool(name="pst", bufs=1, space="PSUM") as pst_pool,
            tc.tile_pool(name="psm", bufs=2, space="PSUM") as psm_pool,
            tc.tile_pool(name="psr", bufs=2, space="PSUM") as psr_pool,
        ):
            w1_sb = cpool.tile([128, 6 * 128], BF16)
            nc.sync.dma_start(w1_sb[:], w1[:])
            w2_sb = cpool.tile([128, 128], BF16)
            nc.sync.dma_start(w2_sb[:], w2[:])
            w3_sb = cpool.tile([128, M0 * SPHERE], BF16)
            nc.sync.dma_start(w3_sb[:], w3[:])
            id_sb = cpool.tile([128, 128], BF16)
            nc.sync.dma_start(id_sb[:], ident[:])

            def layernorm_silu(ps, h_out):
                """h_out = silu(LN(ps)); ps is a [128,128] f32 psum view.
                rsqrt via quake-seeded Newton (2 iters) on DVE+GpSimd so the
                ACT table stays on Silu."""
                st = stat_pool.tile([128, 6], F32, tag="bn")
                nc.vector.bn_stats(st[:], ps)
                mv = stat_pool.tile([128, 2], F32, tag="mv")
                nc.vector.bn_aggr(mv[:], st[:])
                ve = stat_pool.tile([128, 1], F32, tag="ve")
                nc.vector.tensor_scalar(ve[:], mv[:, 1:2], LN_EPS, None,
                                        mybir.AluOpType.add)
                yi = stat_pool.tile([128, 1], I32, tag="yi")
                yf = yi[:].bitcast(F32)
                nc.vector.tensor_scalar(yi[:], ve[:].bitcast(I32), 1, None,
                                        mybir.AluOpType.arith_shift_right)
                nc.vector.tensor_scalar(yi[:], yi[:], -1, RMAGIC,
                                        mybir.AluOpType.mult,
                                        mybir.AluOpType.add)
                t1 = stat_pool.tile([128, 1], F32, tag="t1")
                for _ in range(2):
                    nc.gpsimd.tensor_mul(t1[:], yf, yf)
                    nc.gpsimd.tensor_mul(t1[:], t1[:], ve[:])
                    nc.vector.tensor_scalar(t1[:], t1[:], -0.5, 1.5,
                                            mybir.AluOpType.mult,
                                            mybir.AluOpType.add)
                    nc.gpsimd.tensor_mul(yf, yf, t1[:])
                nm = stat_pool.tile([128, 1], F32, tag="nm")
                nc.gpsimd.tensor_mul(nm[:], mv[:, 0:1], yf)
                nc.vector.tensor_scalar(nm[:], nm[:], -1.0, None,
                                        mybir.AluOpType.mult)
                nc.scalar.activation(h_out[:], ps,
                                     mybir.ActivationFunctionType.Silu,
                                     bias=nm[:], scale=yf)

            for t in range(T):
                is_main = t < T_MAIN
                xw_t = xw_pool.tile([128, XWF], BF16)
                dma(xw_t[:], xw[t])
                if is_main:
                    x_t = x_pool.tile([128, TILE_SLOTS * LFULL], F32)
                    dma(x_t[:], xr[t])

                # MLP layer 1: x_edge @ W1 -> psum [128e, 128ch]
                ps1 = ps_pool.tile([128, 448], F32, tag="ps")
                for k in range(6):
                    nc.tensor.matmul(
                        ps1[:, 0:128],
                        xw_t[:, k * 128:(k + 1) * 128],
                        w1_sb[:, k * 128:(k + 1) * 128],
                        start=(k == 0), stop=(k == 5),
                    )
                h1 = h_pool.tile([128, 128], BF16, tag="h")
                layernorm_silu(ps1[:, 0:128], h1)

                pst1 = pst_pool.tile([128, 128], BF16, tag="pst")
                nc.tensor.transpose(pst1[:], h1[:], id_sb[:])
                h1t = h_pool.tile([128, 128], BF16, tag="ht")
                nc.vector.tensor_copy(h1t[:], pst1[:])

                # MLP layer 2
                ps2 = ps_pool.tile([128, 448], F32, tag="ps")
                nc.tensor.matmul(ps2[:, 0:128], h1t[:], w2_sb[:],
                                 start=True, stop=True)
                h2 = h_pool.tile([128, 128], BF16, tag="h")
                layernorm_silu(ps2[:, 0:128], h2)

                pst2 = pst_pool.tile([128, 128], BF16, tag="pst")
                nc.tensor.transpose(pst2[:], h2[:], id_sb[:])
                h2t = h_pool.tile([128, 128], BF16, tag="ht")
                nc.vector.tensor_copy(h2t[:], pst2[:])

                # MLP layer 3 -> m0 [128e, 896]; cast to bf16 on ACT
                m0a = psm_pool.tile([128, 448], F32, tag="m0")
                nc.tensor.matmul(m0a[:], h2t[:], w3_sb[:, 0:448],
                                 start=True, stop=True)
                m0b = psm_pool.tile([128, 448], F32, tag="m0")
                nc.tensor.matmul(m0b[:], h2t[:], w3_sb[:, 448:896],
                                 start=True, stop=True)
                m0_sb = m0_pool.tile([128, M0 * SPHERE], BF16)
                nc.scalar.activation(m0_sb[:, 0:448], m0a[:],
                                     mybir.ActivationFunctionType.Copy)
                nc.scalar.activation(m0_sb[:, 448:896], m0b[:],
                                     mybir.ActivationFunctionType.Copy)

                # reshape: stacked[7i+m, j, c] = m0[18j+i, 128m+c]
                stacked = st_pool.tile([128, TILE_SLOTS, 128], BF16)
                for j in range(TILE_SLOTS):
                    dma(stacked[0:ROWS, j, :],
                        m0_sb[j * SLOT_E:(j + 1) * SLOT_E, :])

                # rotation, transposed: rotT[c, f] per slot; one psum tile
                rot = psr_pool.tile([128, TILE_SLOTS * LFULL], F32, tag="rot")
                for j in range(TILE_SLOTS):
                    nc.tensor.matmul(
                        rot[:, j * LFULL:(j + 1) * LFULL],
                        stacked[0:ROWS, j, :],
                        xw_t[0:ROWS, 768 + j * LFULL:768 + (j + 1) * LFULL],
                        start=True, stop=True,
                    )
                out_sb = out_pool.tile([128, TILE_SLOTS * LFULL], F32)
                if is_main:
                    nc.vector.tensor_add(out_sb[:], rot[:], x_t[:])
                else:
                    nc.vector.tensor_copy(out_sb[:], rot[:])
                dma(outr[t] if is_main else ovr[t - T_MAIN], out_sb[:])

    return nc


def _envelope(d):
    e = 1.0 + (-21.0) * d ** 5 + 35.0 * d ** 6 + (-15.0) * d ** 7
    return np.where(d < 1.0, e, 0.0)


def kernel(**inputs):
    x = np.asarray(inputs["x"], np.float32)
    dist_emb = np.asarray(inputs["edge_distance_embedding"], np.float32)
    src_emb = np.asarray(inputs["source_atom_embedding"], np.float32)
    tgt_emb = np.asarray(inputs["target_atom_embedding"], np.float32)
    edge_distance = np.asarray(inputs["edge_distance"], np.float64)
    edge_index = np.asarray(inputs["edge_index"]).astype(np.int64)
    wigner = np.asarray(inputs["wigner_and_M_mapping_inv"], np.float32)
    W1 = np.asarray(inputs["W1"], np.float32)
    W2 = np.asarray(inputs["W2"], np.float32)
    W3 = np.asarray(inputs["W3"], np.float32)
    # biases/gains are zeros/ones by construction; folded out of the kernel
    for nm, triv in (("b1", 0), ("bt1", 0), ("b2", 0), ("bt2", 0), ("b3", 0),
                     ("g1", 1), ("g2", 1)):
        v = np.asarray(inputs[nm])
        assert np.all(v == triv), f"{nm} not trivial; unsupported fast path"

    srcs, tgts = edge_index[0], edge_index[1]
    scale = (_envelope(edge_distance / CUTOFF) / RESCALE).astype(np.float32)

    order = np.argsort(tgts, kind="stable")
    tsorted = tgts[order]
    starts = np.searchsorted(tsorted, np.arange(N_NODES + 1))

    # ---- build slots per core ----
    core_slots = []
    max_ov = 0
    for c in range(N_CORES):
        slots_main = []
        slots_ov = []
        base = c * NODES_PER_CORE
        for nl in range(NODES_PER_CORE):
            eids = order[starts[base + nl]:starts[base + nl + 1]]
            slots_main.append((nl, eids[:SLOT_E]))
            rest = eids[SLOT_E:]
            while len(rest) > 0:
                slots_ov.append((nl, rest[:SLOT_E]))
                rest = rest[SLOT_E:]
        for nl in range(NODES_PER_CORE, S_MAIN):
            slots_main.append((nl, np.empty(0, np.int64)))  # dummy
        core_slots.append((slots_main, slots_ov))
        max_ov = max(max_ov, len(slots_ov))

    S_OV = max(TILE_SLOTS, -(-max_ov // TILE_SLOTS) * TILE_SLOTS)
    S = S_MAIN + S_OV
    T = S // TILE_SLOTS
    E_pad = S * SLOT_E

    key = (T, S_OV)
    if key not in _CACHE:
        _CACHE[key] = _build_nc(T, S_OV)
    nc = _CACHE[key]

    # ---- shared weight tensors ----
    w1_in = np.ascontiguousarray(
        W1.reshape(6, 128, 128).transpose(1, 0, 2).reshape(128, 6 * 128)
    ).astype(NP_BF16)
    w2_in = W2.astype(NP_BF16)
    w3_in = W3.astype(NP_BF16)
    ident = np.eye(128, dtype=np.float32).astype(NP_BF16)

    in_maps = []
    ov_maps = []
    for c in range(N_CORES):
        slots_main, slots_ov = core_slots[c]
        slots = slots_main + slots_ov + [
            (0, np.empty(0, np.int64))
        ] * (S_OV - len(slots_ov))

        eorder = np.full(E_pad, -1, np.int64)
        for s, (_, eids) in enumerate(slots):
            eorder[s * SLOT_E:s * SLOT_E + len(eids)] = eids
        valid = eorder >= 0
        idx = eorder[valid]

        # xe gather -> [E_pad, 768]
        xe = np.zeros((E_pad, 768), np.float32)
        xe[valid, :D_DIST] = dist_emb[idx]
        xe[valid, D_DIST:D_DIST + 128] = src_emb[srcs[idx]]
        xe[valid, D_DIST + 128:] = tgt_emb[tgts[idx]]
        # -> [T, 128p, 6k, 128e] with e=126,127 zero
        xeT = np.zeros((T, 128, 6, 128), np.float32)
        xeT[:, :, :, :TILE_E] = (
            xe.reshape(T, TILE_E, 6, 128).transpose(0, 3, 2, 1)
        )

        # wigner stack rows: [S, 128 rows, 49]
        wrows = np.zeros((E_pad, M0, LFULL), np.float32)
        wrows[valid] = (
            wigner[idx, :, :M0] * scale[idx][:, None, None]
        ).transpose(0, 2, 1)
        wrows = wrows.reshape(S, ROWS, LFULL)
        wpad = np.zeros((S, 128, LFULL), np.float32)
        wpad[:, :ROWS] = wrows
        # -> [T, 128 rows, 7 slots, 49]
        wtile = (
            wpad.reshape(T, TILE_SLOTS, 128, LFULL).transpose(0, 2, 1, 3)
        )

        xw_in = np.ascontiguousarray(np.concatenate(
            (xeT.reshape(T, 128, 768), wtile.reshape(T, 128, TILE_SLOTS * LFULL)),
            axis=2,
        )).astype(NP_BF16)

        # x shard, transposed per node: [T_MAIN, 128c, 7j*49f]
        xs = np.zeros((S_MAIN, LFULL, 128), np.float32)
        xs[:NODES_PER_CORE] = x[c * NODES_PER_CORE:(c + 1) * NODES_PER_CORE]
        x_in = np.ascontiguousarray(
            xs.transpose(0, 2, 1)                       # [S_MAIN, 128, 49]
            .reshape(T_MAIN, TILE_SLOTS, 128, LFULL)
            .transpose(0, 2, 1, 3)
            .reshape(T_MAIN, 128, TILE_SLOTS * LFULL)
        )

        in_maps.append({
            "xw": xw_in, "xr": x_in,
            "w1": w1_in, "w2": w2_in, "w3": w3_in, "ident": ident,
        })
        ov_maps.append([nl for nl, _ in slots_ov])

    global LAST
    res = run_bass_kernel_spmd(
        nc, in_maps, core_ids=list(range(N_CORES)), trace=TRACE, **TRACE_KW
    )
    LAST = res

    out = np.empty((N_NODES, LFULL, SPHERE), np.float32)
    for c in range(N_CORES):
        r = res.results[c]
        # [T_MAIN, 128c, 7, 49] -> [S_MAIN, 49, 128]
        o = np.asarray(r["outr"], np.float32).reshape(
            T_MAIN, 128, TILE_SLOTS, LFULL).transpose(0, 2, 3, 1).reshape(
            S_MAIN, LFULL, 128)
        oc = o[:NODES_PER_CORE]
        ov = np.asarray(r["ovr"], np.float32).reshape(
            -1, 128, TILE_SLOTS, LFULL).transpose(0, 2, 3, 1).reshape(
            S_OV, LFULL, 128)
        for s, nl in enumerate(ov_maps[c]):
            oc[nl] += ov[s]
        out[c * NODES_PER_CORE:(c + 1) * NODES_PER_CORE] = oc
    return out
